# revision 6
# baseline (speedup 1.0000x reference)
"""Conv-QKV self-attention (CSA) Trainium2 Bass kernel.

Reference computation (per batch b):
    k = conv1d(x, K_w, K_b); q = conv1d(x, Q_w, Q_b); v = conv1d(x, V_w, V_b)
    scores = relu(k^T q)                # [L, L], contraction over 64 channels
    out = v @ scores / sqrt(3)          # [64, L], contraction over L
Sharding: 8 cores = 4 batches x 2 row-halves (l) of the score matrix.
Each core computes k, v for its l-half, q for the full L, a flash-style
pass over relu(k^T q) tiles, and a PARTIAL out (contraction over its
l-half).  The host sums the two partials per batch.  1/sqrt(3) is folded
into the V weights/bias on the host.

HW model (measured on this TRN2):
 - f32r matmuls with a 512-col moving operand stream at 1 col/cycle
   (213 ns issue-to-issue) -- same rate as bf16, so no row-packing and no
   partition duplication; plain back-to-back matmuls are the roofline.
 - Same-PSUM-bank back-to-back accumulation serializes at ~426 ns, so the
   output accumulation alternates between two PSUM banks (even/odd
   l-tiles) that are summed once per m-tile.
 - Input DMA is the conv-phase critical path: x ships as bf16 (half the
   bytes), in exactly two 2050-col windows (own l-half + other half, each
   with a 1-shifted copy on partitions 64:128 for K-stacked taps 0+1) --
   no duplicated xk/xd traffic.
 - K and V convs share their moving operand, so they run as ONE matmul
   pair per x-tile with a [k|v] 128-col stationary.  v^T is produced by
   8 PE transposes of a [128,128]-packed v buffer.
 - The m (q/out column) axis is processed own-half first so the flash
   loop can start before the other window lands; the host un-permutes.
"""

import numpy as np

FIN, FOUT, KS = 64, 64, 3
B, L = 4, 4096
HALF = L // 2            # per-core l range
NCORES = 8
MT = 512                 # m tile (PSUM bank free dim, fp32)
LT = 128                 # l tile (PE partition dim)
N_MT = L // MT           # 8  (full m range per core)
N_LT = HALF // LT        # 16 (l tiles in this core's half)
WIN = HALF + 2           # x window cols (incl conv halo)
SQRT_KS = float(np.sqrt(KS))

_NC_CACHE = {}


def _build_nc():
    from contextlib import ExitStack

    import concourse.tile as tile
    from concourse import bacc, mybir

    f32 = mybir.dt.float32
    f32r = mybir.dt.float32r
    bf16 = mybir.dt.bfloat16
    AF = mybir.ActivationFunctionType

    nc = bacc.Bacc("TRN2", target_bir_lowering=False)

    # x windows: cols 0:WIN = own l-half (+halo), WIN:2*WIN = other half.
    # rows 0:64 = x[c, col-1], rows 64:128 = x[c, col] (K-stacked taps).
    xw_d = nc.dram_tensor("xw", [128, 2 * WIN], bf16, kind="ExternalInput")
    # fused conv weights: [k taps01 | v taps01] and [k tap2 | v tap2]
    kvw01_d = nc.dram_tensor("kvw01", [128, 128], bf16, kind="ExternalInput")
    kvw2_d = nc.dram_tensor("kvw2", [FIN, 128], bf16, kind="ExternalInput")
    qw01_d = nc.dram_tensor("qw01", [128, FOUT], bf16, kind="ExternalInput")
    qw2_d = nc.dram_tensor("qw2", [FIN, FOUT], bf16, kind="ExternalInput")
    kb_d = nc.dram_tensor("kb", [FOUT, 1], f32, kind="ExternalInput")
    qb_d = nc.dram_tensor("qb", [FOUT, 1], f32, kind="ExternalInput")
    vb_d = nc.dram_tensor("vb", [1, FOUT], f32, kind="ExternalInput")
    id_d = nc.dram_tensor("ident", [128, 128], f32r, kind="ExternalInput")
    out_d = nc.dram_tensor("out", [FOUT, L], f32, kind="ExternalOutput")

    with tile.TileContext(nc) as tc, ExitStack() as ctx:
        consts = ctx.enter_context(tc.tile_pool(name="consts", bufs=1))
        big = ctx.enter_context(tc.tile_pool(name="big", bufs=1))

        # ---- input DMA (sync queue, strict need-order) ----------------
        kvw01_sb = consts.tile([128, 128], bf16)
        nc.sync.dma_start(out=kvw01_sb, in_=kvw01_d[:, :])
        kvw2_sb = consts.tile([FIN, 128], bf16)
        nc.sync.dma_start(out=kvw2_sb, in_=kvw2_d[:, :])
        qw01_sb = consts.tile([128, FOUT], bf16)
        nc.sync.dma_start(out=qw01_sb, in_=qw01_d[:, :])
        qw2_sb = consts.tile([FIN, FOUT], bf16)
        nc.sync.dma_start(out=qw2_sb, in_=qw2_d[:, :])
        kb_sb = consts.tile([FOUT, 1], f32)
        nc.sync.dma_start(out=kb_sb, in_=kb_d[:, :])
        qb_sb = consts.tile([FOUT, 1], f32)
        nc.sync.dma_start(out=qb_sb, in_=qb_d[:, :])
        vb_sb = consts.tile([128, FOUT], f32)
        nc.sync.dma_start(out=vb_sb, in_=vb_d[:, :].to_broadcast([128, FOUT]))
        id_sb = consts.tile([128, 128], f32r)
        nc.sync.dma_start(out=id_sb, in_=id_d[:, :])

        xw_sb = consts.tile([128, 2 * WIN], bf16)
        # chunk boundaries chosen so conv tile g only needs chunks <= g
        bounds = [0, 514, 1026, 1538, WIN]
        for w0 in (0, WIN):  # own window first, then other
            for c in range(4):
                sl = slice(w0 + bounds[c], w0 + bounds[c + 1])
                nc.sync.dma_start(out=xw_sb[:, sl], in_=xw_d[:, sl])

        # ---- big SBUF state -------------------------------------------
        k2_sb = big.tile([FOUT, HALF], f32r)          # k, own half
        q2_sb = big.tile([FOUT, L], f32r)             # q, m-permuted order
        vv_sb = big.tile([128, HALF // 2], f32r)      # v packed 2-high
        vt_sb = big.tile([128, N_LT, FOUT], f32r)     # v^T per l-tile
        warm = consts.tile([FIN, 640], bf16)
        nc.vector.memset(warm, 0.0)

        # ---- stage A: warmup + convs + transposes ---------------------
        actx = ctx.enter_context(ExitStack())
        wpool = actx.enter_context(tc.tile_pool(name="wpsum", bufs=2, space="PSUM"))
        kvpool = actx.enter_context(tc.tile_pool(name="kvpsum", bufs=2, space="PSUM"))
        qapool = actx.enter_context(tc.tile_pool(name="qapsum", bufs=2, space="PSUM"))
        tpool = actx.enter_context(tc.tile_pool(name="tpsum", bufs=2, space="PSUM"))

        # PE warm-up burst while the first DMA chunks stream in (the HAM
        # clock gate keeps the PE slow until it sees sustained activity)
        for i in range(10):
            wp = wpool.tile([128, 256], f32, name="wp", tag="wp")
            nc.tensor.matmul(wp, warm[:, 0:128], warm[:, 384:640], start=True, stop=True)

        def conv_q(src0, dst0, pool):
            # q conv for one 512-col tile: taps01 (K=128) + tap2 (K=64)
            pq = pool.tile([FOUT, MT], f32, name="pq", tag="pq")
            nc.tensor.matmul(pq, qw01_sb, xw_sb[:, src0 : src0 + MT], start=True, stop=False)
            nc.tensor.matmul(pq, qw2_sb, xw_sb[0:FIN, src0 + 2 : src0 + 2 + MT], start=False, stop=True)
            nc.scalar.activation(q2_sb[:, dst0 : dst0 + MT], pq, AF.Identity, bias=qb_sb)

        for g in range(4):  # own-half x tiles
            s0 = g * MT
            pkv = kvpool.tile([128, MT], f32, name="pkv", tag="pkv")
            nc.tensor.matmul(pkv, kvw01_sb, xw_sb[:, s0 : s0 + MT], start=True, stop=False)
            nc.tensor.matmul(pkv, kvw2_sb, xw_sb[0:FIN, s0 + 2 : s0 + 2 + MT], start=False, stop=True)
            conv_q(s0, s0, qapool)
            nc.scalar.activation(k2_sb[:, s0 : s0 + MT], pkv[0:FOUT, :], AF.Identity, bias=kb_sb)
            # v: pack tiles 0,1 on partitions 0:64 and 2,3 on 64:128
            # (vector, not gpsimd: only DVE/Act engines can read PSUM)
            nc.vector.tensor_copy(
                vv_sb[64 * (g // 2) : 64 * (g // 2) + 64, (g % 2) * MT : (g % 2) * MT + MT],
                pkv[FOUT:128, :],
            )

        for t in range(8):  # v^T via PE transpose; v-bias added here
            tp = tpool.tile([128, 128], f32r, name="tp", tag="tp")
            nc.tensor.transpose(tp, vv_sb[:, t * 128 : (t + 1) * 128], id_sb)
            nc.vector.tensor_add(vt_sb[:, t, :], tp[:, 0:FOUT], vb_sb)
            nc.vector.tensor_add(vt_sb[:, t + 8, :], tp[:, FOUT:128], vb_sb)

        actx.close()

        # ---- stage B: flash loop over score tiles ---------------------
        spsum = ctx.enter_context(tc.tile_pool(name="spsum", bufs=2, space="PSUM"))
        opsum = ctx.enter_context(tc.tile_pool(name="opsum", bufs=1, space="PSUM"))
        qbpool = ctx.enter_context(tc.tile_pool(name="qbpsum", bufs=2, space="PSUM"))
        spool = ctx.enter_context(tc.tile_pool(name="spool", bufs=4))
        opool = ctx.enter_context(tc.tile_pool(name="opool", bufs=2))

        NPAIR = N_LT // 2
        for mt in range(N_MT):
            po0 = opsum.tile([FOUT, MT], f32, name="po0", tag="po0")
            po1 = opsum.tile([FOUT, MT], f32, name="po1", tag="po1")
            pending = []  # pairs awaiting their output matmuls

            def flush_mm2(last=False):
                p, ps_sb = pending.pop(0)
                nc.tensor.matmul(
                    po0, vt_sb[:, 2 * p, :], ps_sb[:, 0:MT],
                    start=(p == 0), stop=last,
                )
                nc.tensor.matmul(
                    po1, vt_sb[:, 2 * p + 1, :], ps_sb[:, MT : 2 * MT],
                    start=(p == 0), stop=last,
                )

            for p in range(NPAIR):
                ps = spsum.tile([128, 2 * MT], f32, name="ps")
                nc.tensor.matmul(
                    ps[:, 0:MT],
                    k2_sb[:, (2 * p) * LT : (2 * p + 1) * LT],
                    q2_sb[:, mt * MT : (mt + 1) * MT],
                    start=True, stop=True,
                )
                nc.tensor.matmul(
                    ps[:, MT : 2 * MT],
                    k2_sb[:, (2 * p + 1) * LT : (2 * p + 2) * LT],
                    q2_sb[:, mt * MT : (mt + 1) * MT],
                    start=True, stop=True,
                )
                # software pipeline (depth 2): the output matmuls for pair
                # p-2 issue here, hiding the relu PSUM->SBUF latency
                if len(pending) >= 2:
                    flush_mm2()
                s_sb = spool.tile([128, 2 * MT], f32r, name="s_sb")
                nc.vector.tensor_scalar_max(s_sb[:, 0:MT], ps[:, 0:MT], 0.0)
                nc.scalar.activation(s_sb[:, MT : 2 * MT], ps[:, MT : 2 * MT], AF.Relu)
                pending.append((p, s_sb))

            while pending:
                flush_mm2(last=(len(pending) == 1))
            # combine the two accumulation banks (DVE may read only one
            # PSUM operand, so stage po1 through SBUF on the scalar engine)
            o_tmp = opool.tile([FOUT, MT], f32, name="o_tmp", tag="o_tmp")
            nc.scalar.copy(o_tmp, po1)
            o_sb = opool.tile([FOUT, MT], f32, name="o_sb")
            nc.vector.tensor_add(o_sb, po0, o_tmp)
            nc.gpsimd.dma_start(out_d[:, mt * MT : (mt + 1) * MT], o_sb)

            if mt < 4:  # other-half q conv tile, needed from m-tile 4 on
                conv_q(WIN + mt * MT, HALF + mt * MT, qbpool)

    nc.finalize()
    return nc


def _get_nc():
    if "nc" not in _NC_CACHE:
        _NC_CACHE["nc"] = _build_nc()
    return _NC_CACHE["nc"]


def make_in_maps(x, K_w, K_b, Q_w, Q_b, V_w, V_b):
    """Host-side marshalling: per-core input dicts for the SPMD kernel."""
    import ml_dtypes

    bf16 = ml_dtypes.bfloat16
    x = np.asarray(x, np.float32)
    # xpad col c = x col (c-1); cols 0, L+1, L+2 are zero
    xpad = np.zeros((B, FIN, L + 3), np.float32)
    xpad[:, :, 1 : L + 1] = x

    def wT(w):  # [co, ci, t] -> [t, ci, co]
        return np.transpose(np.asarray(w, np.float32), (2, 1, 0))

    kw, qw = wT(K_w), wT(Q_w)
    vw = wT(V_w) / SQRT_KS
    kvw01 = np.zeros((128, 128), np.float32)
    kvw01[0:64, 0:64], kvw01[64:128, 0:64] = kw[0], kw[1]
    kvw01[0:64, 64:128], kvw01[64:128, 64:128] = vw[0], vw[1]
    kvw2 = np.concatenate([kw[2], vw[2]], axis=1)  # [64, 128]
    qw01 = np.concatenate([qw[0], qw[1]], axis=0)  # [128, 64]
    qw2 = qw[2]
    kb = np.asarray(K_b, np.float32).reshape(FOUT, 1)
    qb = np.asarray(Q_b, np.float32).reshape(FOUT, 1)
    vb = (np.asarray(V_b, np.float32) / SQRT_KS).reshape(1, FOUT)
    ident = np.eye(128, dtype=np.float32)

    cast = lambda a: np.ascontiguousarray(a.astype(bf16))

    def win(b, lo):  # [128, WIN]: x window + 1-shifted copy
        return np.concatenate(
            [xpad[b][:, lo : lo + WIN], xpad[b][:, lo + 1 : lo + WIN + 1]], axis=0
        )

    in_maps = []
    for core in range(NCORES):
        b, h = divmod(core, 2)
        xw = np.concatenate([win(b, h * HALF), win(b, (1 - h) * HALF)], axis=1)
        in_maps.append(
            dict(
                xw=cast(xw), kvw01=cast(kvw01), kvw2=cast(kvw2),
                qw01=cast(qw01), qw2=cast(qw2),
                kb=kb, qb=qb, vb=vb, ident=ident,
            )
        )
    return in_maps


def assemble(results):
    out = np.empty((B, FOUT, L), np.float32)
    for b in range(B):
        o0 = results[2 * b]["out"]        # h=0: m order is already natural
        o1 = results[2 * b + 1]["out"]    # h=1: own half (cols 0:2048) is m>=2048
        out[b] = o0 + np.concatenate([o1[:, HALF:], o1[:, :HALF]], axis=1)
    return out


def kernel(x, K_w, K_b, Q_w, Q_b, V_w, V_b):
    from concourse.bass_utils import run_bass_kernel_spmd

    nc = _get_nc()
    in_maps = make_in_maps(x, K_w, K_b, Q_w, Q_b, V_w, V_b)
    res = run_bass_kernel_spmd(nc, in_maps, core_ids=list(range(NCORES)))
    return assemble(res.results)
